# revision 14
# baseline (speedup 1.0000x reference)
"""MQA attention (16 Q heads, 1 KV head) on 8 trn2 NeuronCores.

Sharding: data-parallel on batch (2) x tensor-parallel on Q heads (4 per
core). Each core computes K/V for its batch (replicated within the batch
group), attention for its 4 heads, and a row-parallel o_proj partial; the
host sums the 4 partials per batch.

Per-core kernel layout strategy: all matmul contractions on partitions.
  xT [1024, 2048] (host pre-transposed)
  qT = wqT.T @ xT -> [256, 2048] as 2 head-pair tiles [128, 2048] (bf16)
  kT duplicated to both partition halves -> row-packed score matmuls
     (K=64 per head, 2 heads share the 128 PE rows)
  scoresT [k, q] per (pair, qchunk, kblock) in PSUM [128, 1024] (2 heads)
  exp: mostly ScalarE (table exp) PSUM->SBUF bf16 with per-key bias;
     a subset of key-blocks computed on DVE via Schraudolph bit-trick
     (i16 = a*s + b; bitcast to bf16) to relieve the ACT bottleneck
  PV: lhsT = [v | ones] bf16 [128, 65] -> attn_outT [64, q] + denom row
  normalize: reciprocal + DMA partition-broadcast + DVE multiply -> bf16
  o_proj: out[q, hidden] partial = attnT.T @ woT (bf16), K=256
"""
import sys

sys.path.insert(0, "/opt/trn_rl_repo")

import ml_dtypes
import numpy as np

import concourse.bass as bass
import concourse.bacc as bacc
import concourse.tile as tile
from concourse import mybir
from concourse.bass_utils import run_bass_kernel_spmd
from concourse.tile_rust import add_dep_helper

HIDDEN = 1024
NH = 16
D = 64
B = 2
S = 2048
NCORES = 8
HEADS_PER_CORE = 4
KB = S // 128   # 16 key blocks
QC = S // 512   # 4 query chunks
P = 128

F32 = mybir.dt.float32
F32R = mybir.dt.float32r
BF16 = mybir.dt.bfloat16
I16 = mybir.dt.int16

# Schraudolph exp on DVE for these key blocks (rest on ScalarE table exp).
DVE_KBS = (5, 11)
SCH_A = 128.0 / float(np.log(2.0))          # 184.664965
# 127*128 centered so the log-error is zero-mean, +0.5 for truncation
SCH_B = 16256.0 - 0.0397 * SCH_A + 0.5      # ~16249.17

_CACHE = {}


def build_kernel():
    nc = bacc.Bacc("TRN2", target_bir_lowering=False, debug=False,
                   num_devices=NCORES)

    xT = nc.dram_tensor("xT", [P, QC, 8, 512], BF16, kind="ExternalInput")
    wqT = nc.dram_tensor("wqT", [HIDDEN, 256], BF16, kind="ExternalInput")
    wkkT = nc.dram_tensor("wkkT", [HIDDEN, 128], BF16, kind="ExternalInput")
    wvT = nc.dram_tensor("wvT", [HIDDEN, 2 * D], BF16, kind="ExternalInput")
    identT = nc.dram_tensor("identT", [P, P], BF16, kind="ExternalInput")
    woT = nc.dram_tensor("woT", [256, HIDDEN], BF16, kind="ExternalInput")
    bias2d = nc.dram_tensor("bias2d", [P, KB], F32, kind="ExternalInput")
    bdve = nc.dram_tensor("bdve", [P, KB], F32, kind="ExternalInput")
    ones2d = nc.dram_tensor("ones2d", [P, KB], BF16, kind="ExternalInput")
    out = nc.dram_tensor("out", [S, HIDDEN], BF16, kind="ExternalOutput")
    # internal DRAM bounce for the per-query 1/denom row broadcast
    bounce = nc.dram_tensor("bounce", [QC, 2, 2, 512], F32)

    with tile.TileContext(nc) as tc:
        with tc.tile_pool(name="persist", bufs=1) as persist:
            xts = [persist.tile([P, 8, 512], BF16, name=f"xt{jj}")
                   for jj in range(QC)]  # per-qchunk xT tiles
            qt = persist.tile([P, 2, S], BF16)          # qT head pairs
            kt = persist.tile([P, S], BF16)             # kT dup both halves
            vaug = persist.tile([P, KB, D + 1], BF16)   # [v | ones]
            attnT_js = [persist.tile([P, 2, 512], BF16, name=f"attnT{jj}")
                        for jj in range(QC)]  # per-j normalized attnT
            wq_sb = persist.tile([P, 8, 256], BF16)
            wkk_sb = persist.tile([P, 8, 128], BF16)
            wv_sb = persist.tile([P, 8, 2 * D], BF16)
            vt_sb = persist.tile([P, S], BF16)
            id_sb = persist.tile([P, P], BF16)
            wo_sb = persist.tile([P, 2, HIDDEN], BF16)
            bias_sb = persist.tile([P, KB], F32)
            bdve_sb = persist.tile([P, KB], F32)

            # ---- input DMAs spread across engine DGE rings so x and the
            # weights stream in parallel and the earliest ring wins.
            nc.scalar.dma_start(out=id_sb, in_=identT[:, :])
            nc.scalar.dma_start(
                out=wq_sb, in_=wqT.ap().rearrange("(kc p) m -> p kc m", p=P))
            xd0 = nc.gpsimd.dma_start(out=xts[0], in_=xT[:, 0, :, :])
            xd1 = nc.sync.dma_start(out=xts[1], in_=xT[:, 1, :, :])
            xd2 = nc.sync.dma_start(out=xts[2], in_=xT[:, 2, :, :])
            xd3 = nc.sync.dma_start(out=xts[3], in_=xT[:, 3, :, :])
            add_dep_helper(xd2.ins, xd1.ins, reason="xt order")
            add_dep_helper(xd3.ins, xd2.ins, reason="xt order")
            nc.scalar.dma_start(
                out=wkk_sb, in_=wkkT.ap().rearrange("(kc p) m -> p kc m", p=P))
            nc.scalar.dma_start(
                out=wv_sb, in_=wvT.ap().rearrange("(kc p) m -> p kc m", p=P))
            for t in range(2):
                nc.gpsimd.dma_start(out=wo_sb[:, t, :],
                                    in_=woT[t * P:(t + 1) * P, :])
            nc.sync.dma_start(out=bias_sb, in_=bias2d[:, :])
            nc.gpsimd.dma_start(out=bdve_sb, in_=bdve[:, :])
            warmup = persist.tile([P, 1], F32)
            nc.scalar.activation(warmup, bias_sb[:, 0:1],
                                 mybir.ActivationFunctionType.Exp)
            nc.sync.dma_start(out=vaug[:, :, D:D + 1], in_=ones2d[:, :])

            # ---- PE warmup: keep the PE p-state ramp going during DMA wait
            with tc.tile_pool(name="warm_ps", bufs=1, space="PSUM") as wps:
                junk = wps.tile([P, P], BF16)
                for _ in range(24):
                    nc.tensor.transpose(junk, id_sb, id_sb)

            # ---- projections (j-major, start as soon as xt_j lands) ----
            with tc.tile_pool(name="proj_ps", bufs=2, space="PSUM") as pps, \
                 tc.tile_pool(name="projq_ps", bufs=3, space="PSUM") as ppsq, \
                 tc.tile_pool(name="projv_ps", bufs=1, space="PSUM") as ppsv:
                for j in range(QC):
                    for pair in range(2):
                        pq = ppsq.tile([P, 512], F32, tag="pq")
                        for kc in range(8):
                            nc.tensor.matmul(
                                pq,
                                lhsT=wq_sb[:, kc, pair * P:(pair + 1) * P],
                                rhs=xts[j][:, kc, :],
                                start=(kc == 0), stop=(kc == 7))
                        nc.vector.tensor_copy(
                            qt[:, pair, j * 512:(j + 1) * 512], pq)
                    pk = pps.tile([P, 512], F32, tag="pk")
                    for kc in range(8):
                        nc.tensor.matmul(
                            pk, lhsT=wkk_sb[:, kc, :],
                            rhs=xts[j][:, kc, :],
                            start=(kc == 0), stop=(kc == 7))
                    nc.vector.tensor_copy(kt[:, j * 512:(j + 1) * 512], pk)
                    pvt = pps.tile([P, 512], F32, tag="pvt")
                    for kc in range(8):
                        nc.tensor.matmul(
                            pvt, lhsT=wv_sb[:, kc, :],
                            rhs=xts[j][:, kc, :],
                            start=(kc == 0), stop=(kc == 7))
                    nc.vector.tensor_copy(vt_sb[:, j * 512:(j + 1) * 512], pvt)
                    for sc in range(4 * j, 4 * j + 4):
                        pv = ppsv.tile([P, D], BF16, tag="pv")
                        nc.tensor.transpose(
                            pv, vt_sb[0:D, sc * P:(sc + 1) * P],
                            id_sb[0:D, 0:D])
                        nc.vector.tensor_copy(vaug[:, sc, 0:D], pv)

            # ---- attention (software-pipelined, LAG units) ----
            # o_proj is interleaved: after qchunk j's norm completes, its
            # o_proj rounds are spread across the following units. o_proj
            # PSUM comes from the score-tile rotation (no extra banks).
            with tc.tile_pool(name="sc_ps", bufs=2, space="PSUM") as scp, \
                 tc.tile_pool(name="att_ps", bufs=2, space="PSUM") as attp, \
                 tc.tile_pool(name="exp_sb", bufs=6) as expp, \
                 tc.tile_pool(name="o_sb", bufs=4) as osb, \
                 tc.tile_pool(name="norm_sb", bufs=3) as normp:
                units = [(j, pair, kb) for j in range(QC)
                         for pair in range(2) for kb in range(KB)]
                LAG = 4
                att_tiles = {}
                ex_store = {}
                oproj_pending = []

                def emit_oproj_block(sc, fast_drain=False):
                    po2 = scp.tile([P, 1024], F32, tag="sc", name=f"po_{sc}")
                    ot = osb.tile([P, 1024], BF16, tag="ot")
                    for n in range(2):
                        for t in range(2):
                            nc.tensor.matmul(
                                po2[:, n * 512:(n + 1) * 512],
                                lhsT=attnT_js[sc // 4][:, t,
                                                       (sc % 4) * P:
                                                       (sc % 4 + 1) * P],
                                rhs=wo_sb[:, t, n * 512:(n + 1) * 512],
                                start=(t == 0), stop=(t == 1))
                    if fast_drain:
                        # final blocks: parallel copies + spread output rings
                        nc.vector.tensor_copy(ot[:, 0:512], po2[:, 0:512])
                        nc.scalar.copy(ot[:, 512:1024], po2[:, 512:1024])
                        eng = (nc.sync, nc.gpsimd, nc.scalar, nc.sync)[sc % 4]
                        eng.dma_start(
                            out=out[sc * P:(sc + 1) * P, :], in_=ot)
                    else:
                        nc.vector.tensor_copy(ot, po2)
                        nc.sync.dma_start(
                            out=out[sc * P:(sc + 1) * P, :], in_=ot)

                def emit_norm(j, pair, attA, attB):
                    for h01, attP in ((0, attA), (1, attB)):
                        tmp = normp.tile([D + 1, 512], F32, tag="tmp")
                        nc.vector.tensor_copy(tmp, attP)  # frees att bank
                        deng = nc.sync if j == QC - 1 else nc.gpsimd
                        ds = normp.tile([D, 8], F32, tag="ds")
                        d0 = deng.dma_start(out=ds, in_=tmp[D:D + 1, :])
                        rs = normp.tile([D, 8], F32, tag="rs")
                        nc.vector.reciprocal(out=rs, in_=ds)
                        bc = normp.tile([D, 1, 512], F32, tag="bc")
                        wdma = deng.dma_start(
                            out=bounce[j, pair, h01, :], in_=rs)
                        rdma = deng.dma_start(
                            out=bc, in_=bounce[j, pair,
                                               h01, :].partition_broadcast(D))
                        add_dep_helper(rdma.ins, wdma.ins, reason="bounce RAW")
                        if h01 == 0:
                            nc.vector.tensor_mul(
                                attnT_js[j][0:D, pair, :],
                                tmp[0:D, :], bc[:, 0, :])
                        else:
                            nt = normp.tile([D, 512], BF16, tag="nt")
                            nc.vector.tensor_mul(nt, tmp[0:D, :], bc[:, 0, :])
                            deng.dma_start(
                                out=attnT_js[j][D:P, pair, :],
                                in_=nt)

                for u in range(len(units) + LAG):
                    if u < len(units):
                        j, pair, kb = units[u]
                        if kb == 0:
                            attA_t = attp.tile([D + 1, 512], F32,
                                               tag="attA", name=f"attA_{u}")
                            attB_t = attp.tile([D + 1, 512], F32,
                                               tag="attB", name=f"attB_{u}")
                            att_tiles[(j, pair)] = (attA_t, attB_t)
                        sc = scp.tile([P, 1024], F32, tag="sc")
                        nc.tensor.matmul(
                            sc[:, 0:512],
                            lhsT=kt[0:D, kb * P:(kb + 1) * P],
                            rhs=qt[0:D, pair, j * 512:(j + 1) * 512],
                            start=True, stop=True)
                        nc.tensor.matmul(
                            sc[:, 512:1024],
                            lhsT=kt[D:P, kb * P:(kb + 1) * P],
                            rhs=qt[D:P, pair, j * 512:(j + 1) * 512],
                            start=True, stop=True)
                        ex = expp.tile([P, 1024], BF16, tag="ex")
                        if kb in DVE_KBS:
                            # Schraudolph: bf16(bitcast i16(round(a*s + b)))
                            nc.vector.tensor_scalar(
                                out=ex.bitcast(I16), in0=sc,
                                scalar1=SCH_A,
                                scalar2=bdve_sb[:, kb:kb + 1],
                                op0=mybir.AluOpType.mult,
                                op1=mybir.AluOpType.add)
                        else:
                            nc.scalar.activation(
                                ex, sc, mybir.ActivationFunctionType.Exp,
                                bias=bias_sb[:, kb:kb + 1], scale=1.0)
                        ex_store[u] = ex
                    if u >= LAG:
                        j2, pair2, kb2 = units[u - LAG]
                        attA, attB = att_tiles[(j2, pair2)]
                        ex2 = ex_store.pop(u - LAG)
                        nc.tensor.matmul(
                            attA, lhsT=vaug[:, kb2, :], rhs=ex2[:, 0:512],
                            start=(kb2 == 0), stop=(kb2 == KB - 1))
                        nc.tensor.matmul(
                            attB, lhsT=vaug[:, kb2, :], rhs=ex2[:, 512:1024],
                            start=(kb2 == 0), stop=(kb2 == KB - 1))
                        if kb2 == KB - 1:
                            emit_norm(j2, pair2, attA, attB)
                            del att_tiles[(j2, pair2)]
                            if pair2 == 1:
                                # delay so the normalize DMA bounce chain
                                # completes before the PE queue reaches the
                                # o_proj matmuls (in-order head-of-line)
                                oproj_pending.extend(
                                    (u + 14, 4 * j2 + b) for b in range(4))
                    # spread one o_proj block per two units
                    if (oproj_pending and u % 2 == 0
                            and u >= oproj_pending[0][0]):
                        emit_oproj_block(oproj_pending.pop(0)[1])
                while oproj_pending:
                    emit_oproj_block(oproj_pending.pop(0)[1], fast_drain=True)

    nc.finalize()
    return nc


def make_in_maps(hidden_states, attention_mask, wq, wk, wv, wo):
    scale = D ** -0.5
    wq_s = (wq * scale).astype(np.float32)
    in_maps = []
    for c in range(NCORES):
        b = c // 4
        g = c % 4
        h0 = g * HEADS_PER_CORE * D  # first row of this core's q heads
        xTt = hidden_states[b].T  # [1024, 2048]
        # [p, j, kc, m] = xT[kc*128+p, j*512+m] -> contiguous 8KB runs/partition
        xTc = np.ascontiguousarray(
            xTt.reshape(8, P, QC, 512).transpose(1, 2, 0, 3))
        wqTc = np.ascontiguousarray(wq_s[h0:h0 + 256, :].T)
        wkkTc = np.ascontiguousarray(
            np.concatenate([wk.T, wk.T], axis=1)).astype(np.float32)
        wvTc = np.ascontiguousarray(np.concatenate([wv.T, wv.T], axis=1))
        woTc = np.ascontiguousarray(wo[:, h0:h0 + 256].T)
        bias = ((1.0 - attention_mask[b]) * -1e30).astype(np.float32)
        bias2d = np.ascontiguousarray(bias.reshape(KB, P).T)
        # Schraudolph intercept column: b + a*bias (clamped so masked keys
        # saturate toward int16 min -> bf16 -0.0)
        bdve = np.maximum(SCH_B + SCH_A * bias2d.astype(np.float64),
                          -60000.0).astype(np.float32)
        in_maps.append({
            "xT": xTc.astype(ml_dtypes.bfloat16),
            "wqT": wqTc.astype(ml_dtypes.bfloat16),
            "wkkT": wkkTc.astype(ml_dtypes.bfloat16),
            "wvT": wvTc.astype(ml_dtypes.bfloat16),
            "identT": np.eye(P).astype(ml_dtypes.bfloat16),
            "woT": woTc.astype(ml_dtypes.bfloat16),
            "bias2d": bias2d,
            "bdve": bdve,
            "ones2d": np.ones((P, KB), dtype=ml_dtypes.bfloat16),
        })
    return in_maps


def run(inputs, trace=False, trace_cores=None):
    """Compile (cached) and run; returns (full_output, BassKernelResults)."""
    if "nc" not in _CACHE:
        _CACHE["nc"] = build_kernel()
    nc = _CACHE["nc"]
    in_maps = make_in_maps(**inputs)
    res = run_bass_kernel_spmd(
        nc, in_maps, list(range(NCORES)), trace=trace,
        trace_cores=trace_cores)
    parts = [res.results[c]["out"] for c in range(NCORES)]
    full = np.empty((B, S, HIDDEN), dtype=np.float32)
    for b in range(B):
        acc = np.zeros((S, HIDDEN), dtype=np.float64)
        for g in range(4):
            acc += parts[4 * b + g]
        full[b] = acc.astype(np.float32)
    return full, res


def kernel(hidden_states, attention_mask, wq, wk, wv, wo):
    full, _ = run(dict(hidden_states=np.asarray(hidden_states),
                       attention_mask=np.asarray(attention_mask),
                       wq=np.asarray(wq), wk=np.asarray(wk),
                       wv=np.asarray(wv), wo=np.asarray(wo)))
    return full


# revision 56
# speedup vs baseline: 1.0772x; 1.0772x over previous
"""MQA attention (16 Q heads, 1 KV head) on 8 trn2 NeuronCores.

Sharding: data-parallel on batch (2) x tensor-parallel on Q heads (4 per
core). Each core computes K/V for its batch (replicated within the batch
group), attention for its 4 heads, and a row-parallel o_proj partial; the
host sums the 4 partials per batch.

Per-core kernel layout strategy: all matmul contractions on partitions.
  xT [1024, 2048] (host pre-transposed)
  qT = wqT.T @ xT -> [256, 2048] as 2 head-pair tiles [128, 2048] (bf16)
  kT duplicated to both partition halves -> row-packed score matmuls
     (K=64 per head, 2 heads share the 128 PE rows)
  scoresT [k, q] per (pair, qchunk, kblock) in PSUM [128, 1024] (2 heads)
  exp: mostly ScalarE (table exp) PSUM->SBUF bf16 with per-key bias;
     a subset of key-blocks computed on DVE via Schraudolph bit-trick
     (i16 = a*s + b; bitcast to bf16) to relieve the ACT bottleneck
  PV: lhsT = [v | ones] bf16 [128, 65] -> attn_outT [64, q] + denom row
  normalize: reciprocal + DMA partition-broadcast + DVE multiply -> bf16
  o_proj: out[q, hidden] partial = attnT.T @ woT (bf16), K=256
"""
import sys

sys.path.insert(0, "/opt/trn_rl_repo")

import ml_dtypes
import numpy as np

import concourse.bass as bass
import concourse.bacc as bacc
import concourse.tile as tile
from concourse import mybir
from concourse.bass_utils import run_bass_kernel_spmd
from concourse.tile_rust import add_dep_helper

HIDDEN = 1024
NH = 16
D = 64
B = 2
S = 2048
NCORES = 8
HEADS_PER_CORE = 4
KB = S // 128   # 16 key blocks
QC = S // 512   # 4 query chunks
P = 128

F32 = mybir.dt.float32
F32R = mybir.dt.float32r
BF16 = mybir.dt.bfloat16
I16 = mybir.dt.int16

# Schraudolph exp on DVE for these key blocks (rest on ScalarE table exp).
DVE_KBS = (2, 5, 8, 11, 14)
SCH_A = 128.0 / float(np.log(2.0))          # 184.664965
# 127*128 centered so the log-error is zero-mean, +0.5 for truncation
SCH_B = 16256.0 - 0.0397 * SCH_A + 0.5      # ~16249.17

_CACHE = {}


def build_kernel():
    nc = bacc.Bacc("TRN2", target_bir_lowering=False, debug=False,
                   num_devices=NCORES)

    xT = nc.dram_tensor("xT", [P, QC, 8, 512], BF16, kind="ExternalInput")
    wqT = nc.dram_tensor("wqT", [HIDDEN, 256], BF16, kind="ExternalInput")
    wkkT = nc.dram_tensor("wkkT", [HIDDEN, 128], BF16, kind="ExternalInput")
    wvT = nc.dram_tensor("wvT", [HIDDEN, 2 * D], BF16, kind="ExternalInput")
    identT = nc.dram_tensor("identT", [P, P], BF16, kind="ExternalInput")
    woT = nc.dram_tensor("woT", [256, HIDDEN], BF16, kind="ExternalInput")
    bias2d = nc.dram_tensor("bias2d", [P, KB], F32, kind="ExternalInput")
    bdve = nc.dram_tensor("bdve", [P, KB], F32, kind="ExternalInput")
    ones2d = nc.dram_tensor("ones2d", [P, KB], BF16, kind="ExternalInput")
    out = nc.dram_tensor("out", [S, HIDDEN], BF16, kind="ExternalOutput")
    # internal DRAM bounce for the per-query 1/denom row broadcast
    bounce = nc.dram_tensor("bounce", [QC, 2, 2, 512], F32)

    with tile.TileContext(nc) as tc:
        with tc.tile_pool(name="persist", bufs=1) as persist:
            xts = [persist.tile([P, 8, 512], BF16, name=f"xt{jj}")
                   for jj in range(QC)]  # per-qchunk xT tiles
            qt = persist.tile([P, 2, S], BF16)          # qT head pairs
            kt = persist.tile([P, S], BF16)             # kT dup both halves
            vaug = persist.tile([P, KB, D + 1], BF16)   # [v | ones]
            attnT_js = [persist.tile([P, 2, 512], BF16, name=f"attnT{jj}")
                        for jj in range(QC)]  # per-j normalized attnT
            wq_sb = persist.tile([P, 8, 256], BF16)
            wkk_sb = persist.tile([P, 8, 128], BF16)
            wv_sb = persist.tile([P, 8, 2 * D], BF16)
            vt_sb = persist.tile([P, S], BF16)
            id_sb = persist.tile([P, P], BF16)
            wo_sb = persist.tile([P, 2, HIDDEN], BF16)
            bias_sb = persist.tile([P, KB], F32)
            bdve_sb = persist.tile([P, KB], F32)

            # ---- input DMAs spread across engine DGE rings so x and the
            # weights stream in parallel and the earliest ring wins.
            nc.scalar.dma_start(out=id_sb, in_=identT[:, :])
            nc.scalar.dma_start(
                out=wq_sb, in_=wqT.ap().rearrange("(kc p) m -> p kc m", p=P))
            xd0 = nc.gpsimd.dma_start(out=xts[0], in_=xT[:, 0, :, :])
            xd1 = nc.sync.dma_start(out=xts[1], in_=xT[:, 1, :, :])
            xd2 = nc.sync.dma_start(out=xts[2], in_=xT[:, 2, :, :])
            xd3 = nc.sync.dma_start(out=xts[3], in_=xT[:, 3, :, :])
            add_dep_helper(xd2.ins, xd1.ins, reason="xt order")
            add_dep_helper(xd3.ins, xd2.ins, reason="xt order")
            nc.scalar.dma_start(
                out=wkk_sb, in_=wkkT.ap().rearrange("(kc p) m -> p kc m", p=P))
            nc.scalar.dma_start(
                out=wv_sb, in_=wvT.ap().rearrange("(kc p) m -> p kc m", p=P))
            for t in range(2):
                nc.gpsimd.dma_start(out=wo_sb[:, t, :],
                                    in_=woT[t * P:(t + 1) * P, :])
            nc.sync.dma_start(out=bias_sb, in_=bias2d[:, :])
            nc.gpsimd.dma_start(out=bdve_sb, in_=bdve[:, :])
            warmup = persist.tile([P, 1], F32)
            nc.scalar.activation(warmup, bias_sb[:, 0:1],
                                 mybir.ActivationFunctionType.Exp)
            nc.sync.dma_start(out=vaug[:, :, D:D + 1], in_=ones2d[:, :])



            # ---- projections (j-major, start as soon as xt_j lands) ----
            with tc.tile_pool(name="proj_ps", bufs=2, space="PSUM") as pps, \
                 tc.tile_pool(name="projq_ps", bufs=3, space="PSUM") as ppsq, \
                 tc.tile_pool(name="projv_ps", bufs=1, space="PSUM") as ppsv:
                for j in range(QC):
                    for pair in range(2):
                        pq = ppsq.tile([P, 512], F32, tag="pq")
                        for kc in range(8):
                            nc.tensor.matmul(
                                pq,
                                lhsT=wq_sb[:, kc, pair * P:(pair + 1) * P],
                                rhs=xts[j][:, kc, :],
                                start=(kc == 0), stop=(kc == 7))
                        nc.vector.tensor_copy(
                            qt[:, pair, j * 512:(j + 1) * 512], pq)
                    pk = pps.tile([P, 512], F32, tag="pk")
                    for kc in range(8):
                        nc.tensor.matmul(
                            pk, lhsT=wkk_sb[:, kc, :],
                            rhs=xts[j][:, kc, :],
                            start=(kc == 0), stop=(kc == 7))
                    nc.vector.tensor_copy(kt[:, j * 512:(j + 1) * 512], pk)
                    pvt = pps.tile([P, 512], F32, tag="pvt")
                    for kc in range(8):
                        nc.tensor.matmul(
                            pvt, lhsT=wv_sb[:, kc, :],
                            rhs=xts[j][:, kc, :],
                            start=(kc == 0), stop=(kc == 7))
                    nc.vector.tensor_copy(vt_sb[:, j * 512:(j + 1) * 512], pvt)
                    for sc in range(4 * j, 4 * j + 4):
                        pv = ppsv.tile([P, D], BF16, tag="pv")
                        nc.tensor.transpose(
                            pv, vt_sb[0:D, sc * P:(sc + 1) * P],
                            id_sb[0:D, 0:D])
                        nc.vector.tensor_copy(vaug[:, sc, 0:D], pv)

            # ---- attention (software-pipelined, LAG units) ----
            # o_proj is interleaved: after qchunk j's norm completes, its
            # o_proj rounds are spread across the following units. o_proj
            # PSUM comes from the score-tile rotation (no extra banks).
            with tc.tile_pool(name="sc_ps", bufs=2, space="PSUM") as scp, \
                 tc.tile_pool(name="att_ps", bufs=2, space="PSUM") as attp, \
                 tc.tile_pool(name="exp_sb", bufs=6) as expp, \
                 tc.tile_pool(name="norm_sb", bufs=4) as normp:
                units = [(j, pair, kb) for j in range(QC)
                         for pair in range(2) for kb in range(KB)]
                LAG = 5
                att_tiles = {}
                ex_store = {}

                norm_stages = []  # (due_u, closure)

                def emit_norm(j, pair, attA, attB, u0):
                    # staged across units so the in-order DVE queue never
                    # blocks on a DMA the stage depends on; last qchunk runs
                    # its DMAs on the idle sync ring ahead of the out DMAs
                    deng = nc.sync if j == QC - 1 else nc.gpsimd
                    st = {}
                    for h01, attP in ((0, attA), (1, attB)):
                        tmp = normp.tile([D + 1, 512], F32, tag="tmp")
                        nc.vector.tensor_copy(tmp, attP)  # frees att bank
                        ds = normp.tile([D, 8], F32, tag="ds")
                        deng.dma_start(out=ds, in_=tmp[D:D + 1, :])
                        st[h01] = (tmp, ds)

                    def s1():
                        for h01 in (0, 1):
                            tmp, ds = st[h01]
                            rs = normp.tile([D, 8], F32, tag="rs")
                            nc.vector.reciprocal(out=rs, in_=ds)
                            wdma = deng.dma_start(
                                out=bounce[j, pair, h01, :], in_=rs)
                            st[h01] = (tmp, wdma)

                    def s2():
                        for h01 in (0, 1):
                            tmp, wdma = st[h01]
                            bc = normp.tile([D, 1, 512], F32, tag="bc")
                            rdma = deng.dma_start(
                                out=bc,
                                in_=bounce[j, pair,
                                           h01, :].partition_broadcast(D))
                            add_dep_helper(rdma.ins, wdma.ins,
                                           reason="bounce RAW")
                            st[h01] = (tmp, bc)

                    def s3():
                        for h01 in (0, 1):
                            tmp, bc = st[h01]
                            if h01 == 0:
                                nc.vector.tensor_mul(
                                    attnT_js[j][0:D, pair, :],
                                    tmp[0:D, :], bc[:, 0, :])
                            else:
                                nt = normp.tile([D, 512], BF16, tag="nt")
                                nc.vector.tensor_mul(nt, tmp[0:D, :],
                                                     bc[:, 0, :])
                                deng.dma_start(
                                    out=attnT_js[j][D:P, pair, :],
                                    in_=nt)

                    norm_stages.append((u0 + 2, s1))
                    norm_stages.append((u0 + 4, s2))
                    norm_stages.append((u0 + 6, s3))

                for u in range(len(units) + LAG):
                    if u < len(units):
                        j, pair, kb = units[u]
                        if kb == 0:
                            attA_t = attp.tile([D + 1, 512], F32,
                                               tag="attA", name=f"attA_{u}")
                            attB_t = attp.tile([D + 1, 512], F32,
                                               tag="attB", name=f"attB_{u}")
                            att_tiles[(j, pair)] = (attA_t, attB_t)
                        sc = scp.tile([P, 1024], F32, tag="sc")
                        nc.tensor.matmul(
                            sc[:, 0:512],
                            lhsT=kt[0:D, kb * P:(kb + 1) * P],
                            rhs=qt[0:D, pair, j * 512:(j + 1) * 512],
                            start=True, stop=True)
                        nc.tensor.matmul(
                            sc[:, 512:1024],
                            lhsT=kt[D:P, kb * P:(kb + 1) * P],
                            rhs=qt[D:P, pair, j * 512:(j + 1) * 512],
                            start=True, stop=True)
                        ex = expp.tile([P, 1024], BF16, tag="ex")
                        if kb in DVE_KBS:
                            # Schraudolph: bf16(bitcast i16(round(a*s + b)))
                            nc.vector.tensor_scalar(
                                out=ex.bitcast(I16), in0=sc,
                                scalar1=SCH_A,
                                scalar2=bdve_sb[:, kb:kb + 1],
                                op0=mybir.AluOpType.mult,
                                op1=mybir.AluOpType.add)
                        else:
                            nc.scalar.activation(
                                ex, sc, mybir.ActivationFunctionType.Exp,
                                bias=bias_sb[:, kb:kb + 1], scale=1.0)
                        ex_store[u] = ex
                    if u >= LAG:
                        j2, pair2, kb2 = units[u - LAG]
                        attA, attB = att_tiles[(j2, pair2)]
                        ex2 = ex_store.pop(u - LAG)
                        nc.tensor.matmul(
                            attA, lhsT=vaug[:, kb2, :], rhs=ex2[:, 0:512],
                            start=(kb2 == 0), stop=(kb2 == KB - 1))
                        nc.tensor.matmul(
                            attB, lhsT=vaug[:, kb2, :], rhs=ex2[:, 512:1024],
                            start=(kb2 == 0), stop=(kb2 == KB - 1))
                        if kb2 == KB - 1:
                            emit_norm(j2, pair2, attA, attB, u)
                            del att_tiles[(j2, pair2)]
                    # run due norm stages (after this unit's own DVE work)
                    while norm_stages and norm_stages[0][0] <= u:
                        norm_stages.pop(0)[1]()
                while norm_stages:
                    norm_stages.pop(0)[1]()

            # ---- o_proj (row-parallel partial, separate phase) ----
            with tc.tile_pool(name="o_ps", bufs=4, space="PSUM") as ops, \
                 tc.tile_pool(name="o_sb", bufs=6) as osb:
                for sc in range(KB):
                    ot = osb.tile([P, 1024], BF16, tag="ot")
                    for n in range(2):
                        po = ops.tile([P, 512], F32, tag="po")
                        for t in range(2):
                            nc.tensor.matmul(
                                po,
                                lhsT=attnT_js[sc // 4][:, t,
                                                       (sc % 4) * P:
                                                       (sc % 4 + 1) * P],
                                rhs=wo_sb[:, t, n * 512:(n + 1) * 512],
                                start=(t == 0), stop=(t == 1))
                        if n == 0:
                            nc.vector.tensor_copy(ot[:, 0:512], po)
                        else:
                            nc.scalar.copy(ot[:, 512:1024], po)
                    eng = (nc.sync, nc.gpsimd, nc.scalar)[sc % 3]
                    eng.dma_start(
                        out=out[sc * P:(sc + 1) * P, :], in_=ot)

    nc.finalize()
    return nc


def make_in_maps(hidden_states, attention_mask, wq, wk, wv, wo):
    scale = D ** -0.5
    wq_s = (wq * scale).astype(np.float32)
    in_maps = []
    for c in range(NCORES):
        b = c // 4
        g = c % 4
        h0 = g * HEADS_PER_CORE * D  # first row of this core's q heads
        xTt = hidden_states[b].T  # [1024, 2048]
        # [p, j, kc, m] = xT[kc*128+p, j*512+m] -> contiguous 8KB runs/partition
        xTc = np.ascontiguousarray(
            xTt.reshape(8, P, QC, 512).transpose(1, 2, 0, 3))
        wqTc = np.ascontiguousarray(wq_s[h0:h0 + 256, :].T)
        wkkTc = np.ascontiguousarray(
            np.concatenate([wk.T, wk.T], axis=1)).astype(np.float32)
        wvTc = np.ascontiguousarray(np.concatenate([wv.T, wv.T], axis=1))
        woTc = np.ascontiguousarray(wo[:, h0:h0 + 256].T)
        bias = ((1.0 - attention_mask[b]) * -1e30).astype(np.float32)
        bias2d = np.ascontiguousarray(bias.reshape(KB, P).T)
        # Schraudolph intercept column: b + a*bias (clamped so masked keys
        # saturate toward int16 min -> bf16 -0.0)
        bdve = np.maximum(SCH_B + SCH_A * bias2d.astype(np.float64),
                          -60000.0).astype(np.float32)
        in_maps.append({
            "xT": xTc.astype(ml_dtypes.bfloat16),
            "wqT": wqTc.astype(ml_dtypes.bfloat16),
            "wkkT": wkkTc.astype(ml_dtypes.bfloat16),
            "wvT": wvTc.astype(ml_dtypes.bfloat16),
            "identT": np.eye(P).astype(ml_dtypes.bfloat16),
            "woT": woTc.astype(ml_dtypes.bfloat16),
            "bias2d": bias2d,
            "bdve": bdve,
            "ones2d": np.ones((P, KB), dtype=ml_dtypes.bfloat16),
        })
    return in_maps


def run(inputs, trace=False, trace_cores=None):
    """Compile (cached) and run; returns (full_output, BassKernelResults)."""
    if "nc" not in _CACHE:
        _CACHE["nc"] = build_kernel()
    nc = _CACHE["nc"]
    in_maps = make_in_maps(**inputs)
    res = run_bass_kernel_spmd(
        nc, in_maps, list(range(NCORES)), trace=trace,
        trace_cores=trace_cores)
    parts = [res.results[c]["out"] for c in range(NCORES)]
    full = np.empty((B, S, HIDDEN), dtype=np.float32)
    for b in range(B):
        acc = np.zeros((S, HIDDEN), dtype=np.float64)
        for g in range(4):
            acc += parts[4 * b + g]
        full[b] = acc.astype(np.float32)
    return full, res


def kernel(hidden_states, attention_mask, wq, wk, wv, wo):
    full, _ = run(dict(hidden_states=np.asarray(hidden_states),
                       attention_mask=np.asarray(attention_mask),
                       wq=np.asarray(wq), wk=np.asarray(wk),
                       wv=np.asarray(wv), wo=np.asarray(wo)))
    return full


# revision 57
# speedup vs baseline: 1.0922x; 1.0140x over previous
"""MQA attention (16 Q heads, 1 KV head) on 8 trn2 NeuronCores.

Sharding: data-parallel on batch (2) x tensor-parallel on Q heads (4 per
core). Each core computes K/V for its batch (replicated within the batch
group), attention for its 4 heads, and a row-parallel o_proj partial; the
host sums the 4 partials per batch.

Per-core kernel layout strategy: all matmul contractions on partitions.
  xT [1024, 2048] (host pre-transposed)
  qT = wqT.T @ xT -> [256, 2048] as 2 head-pair tiles [128, 2048] (bf16)
  kT duplicated to both partition halves -> row-packed score matmuls
     (K=64 per head, 2 heads share the 128 PE rows)
  scoresT [k, q] per (pair, qchunk, kblock) in PSUM [128, 1024] (2 heads)
  exp: mostly ScalarE (table exp) PSUM->SBUF bf16 with per-key bias;
     a subset of key-blocks computed on DVE via Schraudolph bit-trick
     (i16 = a*s + b; bitcast to bf16) to relieve the ACT bottleneck
  PV: lhsT = [v | ones] bf16 [128, 65] -> attn_outT [64, q] + denom row
  normalize: reciprocal + DMA partition-broadcast + DVE multiply -> bf16
  o_proj: out[q, hidden] partial = attnT.T @ woT (bf16), K=256
"""
import sys

sys.path.insert(0, "/opt/trn_rl_repo")

import ml_dtypes
import numpy as np

import concourse.bass as bass
import concourse.bacc as bacc
import concourse.tile as tile
from concourse import mybir
from concourse.bass_utils import run_bass_kernel_spmd
from concourse.tile_rust import add_dep_helper

HIDDEN = 1024
NH = 16
D = 64
B = 2
S = 2048
NCORES = 8
HEADS_PER_CORE = 4
KB = S // 128   # 16 key blocks
QC = S // 512   # 4 query chunks
P = 128

F32 = mybir.dt.float32
F32R = mybir.dt.float32r
BF16 = mybir.dt.bfloat16
I16 = mybir.dt.int16

# Schraudolph exp on DVE for these key blocks (rest on ScalarE table exp).
DVE_KBS = (2, 5, 8, 11, 14)
SCH_A = 128.0 / float(np.log(2.0))          # 184.664965
# 127*128 centered so the log-error is zero-mean, +0.5 for truncation
SCH_B = 16256.0 - 0.0397 * SCH_A + 0.5      # ~16249.17

_CACHE = {}


def build_kernel():
    nc = bacc.Bacc("TRN2", target_bir_lowering=False, debug=False,
                   num_devices=NCORES)

    xT = nc.dram_tensor("xT", [P, QC, 8, 512], BF16, kind="ExternalInput")
    wqT = nc.dram_tensor("wqT", [HIDDEN, 256], BF16, kind="ExternalInput")
    wkkT = nc.dram_tensor("wkkT", [HIDDEN, 128], BF16, kind="ExternalInput")
    wvT = nc.dram_tensor("wvT", [HIDDEN, 2 * D], BF16, kind="ExternalInput")
    identT = nc.dram_tensor("identT", [P, P], BF16, kind="ExternalInput")
    woT = nc.dram_tensor("woT", [256, HIDDEN], BF16, kind="ExternalInput")
    bias2d = nc.dram_tensor("bias2d", [P, KB], F32, kind="ExternalInput")
    bdve = nc.dram_tensor("bdve", [P, KB], F32, kind="ExternalInput")
    ones2d = nc.dram_tensor("ones2d", [P, KB], BF16, kind="ExternalInput")
    out = nc.dram_tensor("out", [S, HIDDEN], BF16, kind="ExternalOutput")
    # internal DRAM bounce for the per-query 1/denom row broadcast
    bounce = nc.dram_tensor("bounce", [QC, 2, 2, 512], F32)

    with tile.TileContext(nc) as tc:
        with tc.tile_pool(name="persist", bufs=1) as persist:
            xts = [persist.tile([P, 8, 512], BF16, name=f"xt{jj}")
                   for jj in range(QC)]  # per-qchunk xT tiles
            qt = persist.tile([P, 2, S], BF16)          # qT head pairs
            kt = persist.tile([P, S], BF16)             # kT dup both halves
            vaug = persist.tile([P, KB, D + 1], BF16)   # [v | ones]
            attnT_js = [persist.tile([P, 2, 512], BF16, name=f"attnT{jj}")
                        for jj in range(QC)]  # per-j normalized attnT
            wq_sb = persist.tile([P, 8, 256], BF16)
            wkk_sb = persist.tile([P, 8, 128], BF16)
            wv_sb = persist.tile([P, 8, 2 * D], BF16)
            vt_sb = persist.tile([P, S], BF16)
            id_sb = persist.tile([P, P], BF16)
            wo_sb = persist.tile([P, 2, HIDDEN], BF16)
            bias_sb = persist.tile([P, KB], F32)
            bdve_sb = persist.tile([P, KB], F32)

            # ---- input DMAs spread across engine DGE rings so x and the
            # weights stream in parallel and the earliest ring wins. wq and
            # x0 are chunked so the first q-proj matmul (which needs only
            # kc-chunk 0) starts as soon as the first slices land.
            wq_ap = wqT.ap().rearrange("(kc p) m -> p kc m", p=P)
            for kc in range(8):
                eng = nc.scalar if kc % 2 == 0 else nc.sync
                eng.dma_start(out=wq_sb[:, kc, :], in_=wq_ap[:, kc, :])
            xd0a = nc.gpsimd.dma_start(out=xts[0][:, 0:4, :],
                                       in_=xT[:, 0, 0:4, :])
            xd0b = nc.gpsimd.dma_start(out=xts[0][:, 4:8, :],
                                       in_=xT[:, 0, 4:8, :])
            xd1 = nc.sync.dma_start(out=xts[1], in_=xT[:, 1, :, :])
            xd2 = nc.sync.dma_start(out=xts[2], in_=xT[:, 2, :, :])
            xd3 = nc.sync.dma_start(out=xts[3], in_=xT[:, 3, :, :])
            add_dep_helper(xd2.ins, xd1.ins, reason="xt order")
            add_dep_helper(xd3.ins, xd2.ins, reason="xt order")
            nc.scalar.dma_start(out=id_sb, in_=identT[:, :])
            wkk_ap = wkkT.ap().rearrange("(kc p) m -> p kc m", p=P)
            wv_ap = wvT.ap().rearrange("(kc p) m -> p kc m", p=P)
            for h in range(2):
                sl = slice(4 * h, 4 * h + 4)
                nc.scalar.dma_start(out=wkk_sb[:, sl, :], in_=wkk_ap[:, sl, :])
                nc.scalar.dma_start(out=wv_sb[:, sl, :], in_=wv_ap[:, sl, :])
            for t in range(2):
                nc.gpsimd.dma_start(out=wo_sb[:, t, :],
                                    in_=woT[t * P:(t + 1) * P, :])
            nc.sync.dma_start(out=bias_sb, in_=bias2d[:, :])
            nc.gpsimd.dma_start(out=bdve_sb, in_=bdve[:, :])
            warmup = persist.tile([P, 1], F32)
            nc.scalar.activation(warmup, bias_sb[:, 0:1],
                                 mybir.ActivationFunctionType.Exp)
            nc.sync.dma_start(out=vaug[:, :, D:D + 1], in_=ones2d[:, :])



            # ---- projections (j-major, start as soon as xt_j lands) ----
            with tc.tile_pool(name="proj_ps", bufs=2, space="PSUM") as pps, \
                 tc.tile_pool(name="projq_ps", bufs=3, space="PSUM") as ppsq, \
                 tc.tile_pool(name="projv_ps", bufs=1, space="PSUM") as ppsv:
                for j in range(QC):
                    for pair in range(2):
                        pq = ppsq.tile([P, 512], F32, tag="pq")
                        for kc in range(8):
                            nc.tensor.matmul(
                                pq,
                                lhsT=wq_sb[:, kc, pair * P:(pair + 1) * P],
                                rhs=xts[j][:, kc, :],
                                start=(kc == 0), stop=(kc == 7))
                        nc.vector.tensor_copy(
                            qt[:, pair, j * 512:(j + 1) * 512], pq)
                    pk = pps.tile([P, 512], F32, tag="pk")
                    for kc in range(8):
                        nc.tensor.matmul(
                            pk, lhsT=wkk_sb[:, kc, :],
                            rhs=xts[j][:, kc, :],
                            start=(kc == 0), stop=(kc == 7))
                    nc.vector.tensor_copy(kt[:, j * 512:(j + 1) * 512], pk)
                    pvt = pps.tile([P, 512], F32, tag="pvt")
                    for kc in range(8):
                        nc.tensor.matmul(
                            pvt, lhsT=wv_sb[:, kc, :],
                            rhs=xts[j][:, kc, :],
                            start=(kc == 0), stop=(kc == 7))
                    nc.vector.tensor_copy(vt_sb[:, j * 512:(j + 1) * 512], pvt)
                    for sc in range(4 * j, 4 * j + 4):
                        pv = ppsv.tile([P, D], BF16, tag="pv")
                        nc.tensor.transpose(
                            pv, vt_sb[0:D, sc * P:(sc + 1) * P],
                            id_sb[0:D, 0:D])
                        nc.vector.tensor_copy(vaug[:, sc, 0:D], pv)

            # ---- attention (software-pipelined, LAG units) ----
            # o_proj is interleaved: after qchunk j's norm completes, its
            # o_proj rounds are spread across the following units. o_proj
            # PSUM comes from the score-tile rotation (no extra banks).
            with tc.tile_pool(name="sc_ps", bufs=2, space="PSUM") as scp, \
                 tc.tile_pool(name="att_ps", bufs=2, space="PSUM") as attp, \
                 tc.tile_pool(name="exp_sb", bufs=6) as expp, \
                 tc.tile_pool(name="norm_sb", bufs=4) as normp:
                units = [(j, pair, kb) for j in range(QC)
                         for pair in range(2) for kb in range(KB)]
                LAG = 5
                att_tiles = {}
                ex_store = {}

                norm_stages = []  # (due_u, closure)

                def emit_norm(j, pair, attA, attB, u0):
                    # staged across units so the in-order DVE queue never
                    # blocks on a DMA the stage depends on; last qchunk runs
                    # its DMAs on the idle sync ring ahead of the out DMAs
                    deng = nc.sync if j == QC - 1 else nc.gpsimd
                    st = {}
                    for h01, attP in ((0, attA), (1, attB)):
                        tmp = normp.tile([D + 1, 512], F32, tag="tmp")
                        nc.vector.tensor_copy(tmp, attP)  # frees att bank
                        ds = normp.tile([D, 8], F32, tag="ds")
                        deng.dma_start(out=ds, in_=tmp[D:D + 1, :])
                        st[h01] = (tmp, ds)

                    def s1():
                        for h01 in (0, 1):
                            tmp, ds = st[h01]
                            rs = normp.tile([D, 8], F32, tag="rs")
                            nc.vector.reciprocal(out=rs, in_=ds)
                            wdma = deng.dma_start(
                                out=bounce[j, pair, h01, :], in_=rs)
                            st[h01] = (tmp, wdma)

                    def s2():
                        for h01 in (0, 1):
                            tmp, wdma = st[h01]
                            bc = normp.tile([D, 1, 512], F32, tag="bc")
                            rdma = deng.dma_start(
                                out=bc,
                                in_=bounce[j, pair,
                                           h01, :].partition_broadcast(D))
                            add_dep_helper(rdma.ins, wdma.ins,
                                           reason="bounce RAW")
                            st[h01] = (tmp, bc)

                    def s3():
                        for h01 in (0, 1):
                            tmp, bc = st[h01]
                            if h01 == 0:
                                nc.vector.tensor_mul(
                                    attnT_js[j][0:D, pair, :],
                                    tmp[0:D, :], bc[:, 0, :])
                            else:
                                nt = normp.tile([D, 512], BF16, tag="nt")
                                nc.vector.tensor_mul(nt, tmp[0:D, :],
                                                     bc[:, 0, :])
                                deng.dma_start(
                                    out=attnT_js[j][D:P, pair, :],
                                    in_=nt)

                    norm_stages.append((u0 + 2, s1))
                    norm_stages.append((u0 + 4, s2))
                    norm_stages.append((u0 + 6, s3))

                for u in range(len(units) + LAG):
                    if u < len(units):
                        j, pair, kb = units[u]
                        if kb == 0:
                            attA_t = attp.tile([D + 1, 512], F32,
                                               tag="attA", name=f"attA_{u}")
                            attB_t = attp.tile([D + 1, 512], F32,
                                               tag="attB", name=f"attB_{u}")
                            att_tiles[(j, pair)] = (attA_t, attB_t)
                        sc = scp.tile([P, 1024], F32, tag="sc")
                        nc.tensor.matmul(
                            sc[:, 0:512],
                            lhsT=kt[0:D, kb * P:(kb + 1) * P],
                            rhs=qt[0:D, pair, j * 512:(j + 1) * 512],
                            start=True, stop=True)
                        nc.tensor.matmul(
                            sc[:, 512:1024],
                            lhsT=kt[D:P, kb * P:(kb + 1) * P],
                            rhs=qt[D:P, pair, j * 512:(j + 1) * 512],
                            start=True, stop=True)
                        ex = expp.tile([P, 1024], BF16, tag="ex")
                        if kb in DVE_KBS:
                            # Schraudolph: bf16(bitcast i16(round(a*s + b)))
                            nc.vector.tensor_scalar(
                                out=ex.bitcast(I16), in0=sc,
                                scalar1=SCH_A,
                                scalar2=bdve_sb[:, kb:kb + 1],
                                op0=mybir.AluOpType.mult,
                                op1=mybir.AluOpType.add)
                        else:
                            nc.scalar.activation(
                                ex, sc, mybir.ActivationFunctionType.Exp,
                                bias=bias_sb[:, kb:kb + 1], scale=1.0)
                        ex_store[u] = ex
                    if u >= LAG:
                        j2, pair2, kb2 = units[u - LAG]
                        attA, attB = att_tiles[(j2, pair2)]
                        ex2 = ex_store.pop(u - LAG)
                        nc.tensor.matmul(
                            attA, lhsT=vaug[:, kb2, :], rhs=ex2[:, 0:512],
                            start=(kb2 == 0), stop=(kb2 == KB - 1))
                        nc.tensor.matmul(
                            attB, lhsT=vaug[:, kb2, :], rhs=ex2[:, 512:1024],
                            start=(kb2 == 0), stop=(kb2 == KB - 1))
                        if kb2 == KB - 1:
                            emit_norm(j2, pair2, attA, attB, u)
                            del att_tiles[(j2, pair2)]
                    # run due norm stages (after this unit's own DVE work)
                    while norm_stages and norm_stages[0][0] <= u:
                        norm_stages.pop(0)[1]()
                while norm_stages:
                    norm_stages.pop(0)[1]()

            # ---- o_proj (row-parallel partial, separate phase) ----
            with tc.tile_pool(name="o_ps", bufs=4, space="PSUM") as ops, \
                 tc.tile_pool(name="o_sb", bufs=6) as osb:
                for sc in range(KB):
                    ot = osb.tile([P, 1024], BF16, tag="ot")
                    for n in range(2):
                        po = ops.tile([P, 512], F32, tag="po")
                        for t in range(2):
                            nc.tensor.matmul(
                                po,
                                lhsT=attnT_js[sc // 4][:, t,
                                                       (sc % 4) * P:
                                                       (sc % 4 + 1) * P],
                                rhs=wo_sb[:, t, n * 512:(n + 1) * 512],
                                start=(t == 0), stop=(t == 1))
                        if n == 0:
                            nc.vector.tensor_copy(ot[:, 0:512], po)
                        else:
                            nc.scalar.copy(ot[:, 512:1024], po)
                    eng = (nc.sync, nc.gpsimd, nc.scalar)[sc % 3]
                    eng.dma_start(
                        out=out[sc * P:(sc + 1) * P, :], in_=ot)

    nc.finalize()
    return nc


def make_in_maps(hidden_states, attention_mask, wq, wk, wv, wo):
    scale = D ** -0.5
    wq_s = (wq * scale).astype(np.float32)
    in_maps = []
    for c in range(NCORES):
        b = c // 4
        g = c % 4
        h0 = g * HEADS_PER_CORE * D  # first row of this core's q heads
        xTt = hidden_states[b].T  # [1024, 2048]
        # [p, j, kc, m] = xT[kc*128+p, j*512+m] -> contiguous 8KB runs/partition
        xTc = np.ascontiguousarray(
            xTt.reshape(8, P, QC, 512).transpose(1, 2, 0, 3))
        wqTc = np.ascontiguousarray(wq_s[h0:h0 + 256, :].T)
        wkkTc = np.ascontiguousarray(
            np.concatenate([wk.T, wk.T], axis=1)).astype(np.float32)
        wvTc = np.ascontiguousarray(np.concatenate([wv.T, wv.T], axis=1))
        woTc = np.ascontiguousarray(wo[:, h0:h0 + 256].T)
        bias = ((1.0 - attention_mask[b]) * -1e30).astype(np.float32)
        bias2d = np.ascontiguousarray(bias.reshape(KB, P).T)
        # Schraudolph intercept column: b + a*bias (clamped so masked keys
        # saturate toward int16 min -> bf16 -0.0)
        bdve = np.maximum(SCH_B + SCH_A * bias2d.astype(np.float64),
                          -60000.0).astype(np.float32)
        in_maps.append({
            "xT": xTc.astype(ml_dtypes.bfloat16),
            "wqT": wqTc.astype(ml_dtypes.bfloat16),
            "wkkT": wkkTc.astype(ml_dtypes.bfloat16),
            "wvT": wvTc.astype(ml_dtypes.bfloat16),
            "identT": np.eye(P).astype(ml_dtypes.bfloat16),
            "woT": woTc.astype(ml_dtypes.bfloat16),
            "bias2d": bias2d,
            "bdve": bdve,
            "ones2d": np.ones((P, KB), dtype=ml_dtypes.bfloat16),
        })
    return in_maps


def run(inputs, trace=False, trace_cores=None):
    """Compile (cached) and run; returns (full_output, BassKernelResults)."""
    if "nc" not in _CACHE:
        _CACHE["nc"] = build_kernel()
    nc = _CACHE["nc"]
    in_maps = make_in_maps(**inputs)
    res = run_bass_kernel_spmd(
        nc, in_maps, list(range(NCORES)), trace=trace,
        trace_cores=trace_cores)
    parts = [res.results[c]["out"] for c in range(NCORES)]
    full = np.empty((B, S, HIDDEN), dtype=np.float32)
    for b in range(B):
        acc = np.zeros((S, HIDDEN), dtype=np.float64)
        for g in range(4):
            acc += parts[4 * b + g]
        full[b] = acc.astype(np.float32)
    return full, res


def kernel(hidden_states, attention_mask, wq, wk, wv, wo):
    full, _ = run(dict(hidden_states=np.asarray(hidden_states),
                       attention_mask=np.asarray(attention_mask),
                       wq=np.asarray(wq), wk=np.asarray(wk),
                       wv=np.asarray(wv), wo=np.asarray(wo)))
    return full
